# revision 9
# baseline (speedup 1.0000x reference)
"""Trainium2 Bass kernel for a 6-layer GPT forward pass (BigramLanguageModel).

Strategy: data-parallel over batch across 8 NeuronCores (16 samples/core),
no collectives.  Host does the embedding gather (pure input prep) and the
final loss reduction from the logits the device produces.

Device-side layout: residual stream x kept row-major [tokens, D] in SBUF
(exact fp32); LayerNorm outputs are PE-transposed to feature-major [D, tok]
chunks; q/k are produced directly transposed via matmul orientation;
attention scores are computed transposed [Tk, Tq] so softmax denominators
come for free from a ones-column appended to V; all matmuls in float32r
(1 cycle/row at N>=256).
"""
import sys
import numpy as np

sys.path.insert(0, '/opt/trn_rl_repo')

V, D, T, H, HS, L, B, FF = 65, 384, 256, 6, 64, 6, 128, 1536
EPS = 1e-5
SCALE = D ** -0.5
NCORES = 8
BL = B // NCORES            # local batch per core
R = BL * T                  # rows (tokens) per core
RTILES = R // 128           # 32 row tiles
NEG = -30.0                 # additive causal mask value (exp(-30) ~= 1e-13)

_CACHE = {}


def _build_nc(nlayers=L, do_attn=True, do_ff=True):
    import concourse.bacc as bacc
    import concourse.tile as tile
    from concourse import mybir
    from concourse.bass import ts

    F32 = mybir.dt.float32
    F32R = mybir.dt.float32r
    AF = mybir.ActivationFunctionType
    ALU = mybir.AluOpType

    nc = bacc.Bacc()

    x0_d = nc.declare_dram_parameter("x0", [R, D], F32, isOutput=False)
    wq_d = nc.declare_dram_parameter("wq", [L, D, D], F32R, isOutput=False)
    wk_d = nc.declare_dram_parameter("wk", [L, D, D], F32R, isOutput=False)
    wv_d = nc.declare_dram_parameter("wv", [L, D, D], F32R, isOutput=False)
    wp_d = nc.declare_dram_parameter("wp", [L, D, D], F32R, isOutput=False)
    w1_d = nc.declare_dram_parameter("w1", [L, D, FF], F32R, isOutput=False)
    w2_d = nc.declare_dram_parameter("w2", [L, FF, D], F32R, isOutput=False)
    b1_d = nc.declare_dram_parameter("b1", [L, FF], F32, isOutput=False)
    wlm_d = nc.declare_dram_parameter("wlm", [D, V], F32R, isOutput=False)
    em_d = nc.declare_dram_parameter("emat", [H, D], F32R, isOutput=False)
    id_d = nc.declare_dram_parameter("ident", [128, 128], F32R, isOutput=False)
    mk_d = nc.declare_dram_parameter("masks", [128, 2, 256], F32, isOutput=False)
    on_d = nc.declare_dram_parameter("vones", [128, 2 * H], F32R, isOutput=False)
    lg_d = nc.declare_dram_parameter("logitsT", [V, R], F32, isOutput=True)

    with tile.TileContext(nc) as tc:
        with (
            tc.tile_pool(name="xres", bufs=1) as xres,
            tc.tile_pool(name="const", bufs=1) as cpool,
            tc.tile_pool(name="wqkv", bufs=2) as wqkv,
            tc.tile_pool(name="wff", bufs=1) as wff,
            tc.tile_pool(name="work", bufs=2) as work,
            tc.tile_pool(name="small", bufs=4) as small,
            tc.tile_pool(name="ps256", bufs=3, space="PSUM") as ps256,
            tc.tile_pool(name="ps384", bufs=2, space="PSUM") as ps384,
            tc.tile_pool(name="psO", bufs=1, space="PSUM") as psOp,
        ):
            # ---- constants ----
            ident = cpool.tile([128, 128], F32R)
            nc.sync.dma_start(ident, id_d[:, :])
            masks = cpool.tile([128, 2, 256], F32)
            nc.sync.dma_start(masks, mk_d[:, :, :])
            emat = cpool.tile([H, D], F32R)
            nc.sync.dma_start(emat, em_d[:, :])
            vones = cpool.tile([128, 2 * H], F32R)
            nc.sync.dma_start(vones, on_d[:, :])
            wlm_sb = cpool.tile([128, 3, V], F32R)
            nc.sync.dma_start(wlm_sb, wlm_d.rearrange("(kt p) n -> p kt n", p=128))
            eps_t = cpool.tile([128, 1], F32)
            nc.vector.memset(eps_t, EPS)

            # ---- residual stream ----
            x0r = x0_d.rearrange("(n p) d -> n p d", p=128)
            xt = []
            for i in range(RTILES):
                xtile = xres.tile([128, D], F32, tag=f"x{i}")
                nc.sync.dma_start(xtile, x0r[i])
                xt.append(xtile)

            def layernorm(x_ap, out_ap):
                """LN (gain=1, bias=0) row-major; writes f32r."""
                st = small.tile([128, 6], F32, tag="bnst")
                nc.vector.bn_stats(out=st, in_=x_ap)
                mv = small.tile([128, 2], F32, tag="bnmv")
                nc.vector.bn_aggr(out=mv, in_=st)
                std = small.tile([128, 1], F32, tag="std")
                nc.scalar.activation(out=std, in_=mv[:, 1:2], func=AF.Sqrt,
                                     bias=eps_t, scale=1.0)
                rstd = small.tile([128, 1], F32, tag="rstd")
                nc.vector.reciprocal(out=rstd, in_=std)
                with nc.allow_low_precision(reason="f32r matmul inputs"):
                    nc.vector.tensor_scalar(
                        out=out_ap, in0=x_ap, scalar1=mv[:, 0:1], scalar2=rstd,
                        op0=ALU.subtract, op1=ALU.mult)

            def ln_transpose(i0, i1, tag="hT"):
                """LN two row tiles and produce transposed [D, 256] chunk."""
                hT = work.tile([128, 3, 256], F32R, tag=tag)
                h0 = work.tile([128, D], F32R, tag="htmp")
                layernorm(xt[i0], h0)
                h1 = work.tile([128, D], F32R, tag="htmp")
                layernorm(xt[i1], h1)
                for g in range(3):
                    pt = ps256.tile([128, 256], F32R, tag="ps256")
                    nc.tensor.transpose(pt[:, 0:128], h0[:, ts(g, 128)], ident)
                    nc.tensor.transpose(pt[:, 128:256], h1[:, ts(g, 128)], ident)
                    nc.scalar.activation(out=hT[:, g, :], in_=pt, func=AF.Copy)
                return hT

            def attn_block(i0, i1, wq_sb, wk_sb, wv_sb, wp_sb):
                hT = ln_transpose(i0, i1)
                qT = work.tile([128, 3, 256], F32R, tag="qT")
                kT = work.tile([128, 3, 256], F32R, tag="kT")
                for dst, w_sb in ((qT, wq_sb), (kT, wk_sb)):
                    for g in range(3):
                        pq = ps256.tile([128, 256], F32, tag="ps256")
                        for kt in range(3):
                            nc.tensor.matmul(pq, lhsT=w_sb[:, kt, ts(g, 128)],
                                             rhs=hT[:, kt, :],
                                             start=(kt == 0), stop=(kt == 2))
                        nc.any.tensor_copy(dst[:, g, :], pq)
                vaug = work.tile([128, 2, H, 65], F32R, tag="vaug")
                nc.vector.tensor_copy(
                    vaug[:, :, :, 64:65].rearrange("p a h o -> p (a h o)"), vones)
                for tt in range(2):
                    pv = ps384.tile([128, D], F32, tag="ps384")
                    for kt in range(3):
                        nc.tensor.matmul(pv, lhsT=hT[:, kt, ts(tt, 128)],
                                         rhs=wv_sb[:, kt, :],
                                         start=(kt == 0), stop=(kt == 2))
                    nc.any.tensor_copy(
                        vaug[:, tt, :, 0:64],
                        pv.rearrange("p (h e) -> p h e", e=64))

                psO = psOp.tile([V, H, 256], F32, tag="psO")
                for h in range(H):
                    g, r0 = h // 2, 64 * (h % 2)
                    kT_h = kT[r0:r0 + 64, g, :]
                    qT_h = qT[r0:r0 + 64, g, :]
                    sT = work.tile([128, 2, 256], F32R, tag="sT")
                    for kt in range(2):
                        pst = ps256.tile([128, 256], F32, tag="ps256")
                        nc.tensor.matmul(pst, lhsT=kT_h[:, ts(kt, 128)],
                                         rhs=qT_h, start=True, stop=True)
                        with nc.allow_low_precision(reason="softmax f32r"):
                            nc.vector.tensor_add(sT[:, kt, :], pst,
                                                 masks[:, kt, :])
                        nc.scalar.activation(out=sT[:, kt, :],
                                             in_=sT[:, kt, :], func=AF.Exp)
                    for kt in range(2):
                        nc.tensor.matmul(psO[:, h, :],
                                         lhsT=vaug[:, kt, h, :],
                                         rhs=sT[:, kt, :],
                                         start=(kt == 0), stop=(kt == 1))

                # denominators -> reciprocal -> broadcast via E-matmul
                rec1 = work.tile([1, H * 256], F32R, tag="rec1")
                with nc.allow_low_precision(reason="softmax denom"):
                    nc.vector.reciprocal(
                        out=rec1,
                        in_=psO[64:65, :, :].rearrange("o h t -> o (h t)"))
                rec = work.tile([H, 256], F32R, tag="rec")
                nc.sync.dma_start(rec, rec1[:, :])
                oT = work.tile([128, 3, 256], F32R, tag="oT")
                for g in range(3):
                    pbc = ps256.tile([128, 256], F32, tag="ps256")
                    nc.tensor.matmul(pbc, lhsT=emat[:, ts(g, 128)], rhs=rec,
                                     start=True, stop=True)
                    bc = work.tile([128, 256], F32, tag="bc")
                    nc.scalar.activation(out=bc, in_=pbc, func=AF.Copy)
                    with nc.allow_low_precision(reason="attn out f32r"):
                        nc.vector.tensor_mul(oT[0:64, g, :],
                                             psO[0:64, 2 * g, :], bc[0:64, :])
                        nc.vector.tensor_mul(oT[64:128, g, :],
                                             psO[0:64, 2 * g + 1, :],
                                             bc[64:128, :])

                # projection + residual
                for rt, it in ((0, i0), (1, i1)):
                    pxp = ps384.tile([128, D], F32, tag="ps384")
                    for g in range(3):
                        nc.tensor.matmul(pxp, lhsT=oT[:, g, ts(rt, 128)],
                                         rhs=wp_sb[:, g, :],
                                         start=(g == 0), stop=(g == 2))
                    nc.vector.tensor_add(xt[it], xt[it], pxp)

            def ff_block(i0, i1, w1_sb, w2_sb, b1_sb):
                h2T = ln_transpose(i0, i1)
                ff1 = work.tile([128, 12, 256], F32R, tag="ff1")
                for m in range(12):
                    pf = ps256.tile([128, 256], F32, tag="ps256")
                    for kt in range(3):
                        nc.tensor.matmul(pf, lhsT=w1_sb[:, kt, ts(m, 128)],
                                         rhs=h2T[:, kt, :],
                                         start=(kt == 0), stop=(kt == 2))
                    nc.scalar.activation(out=ff1[:, m, :], in_=pf, func=AF.Relu,
                                         bias=b1_sb[:, m:m + 1], scale=1.0)
                for rt, it in ((0, i0), (1, i1)):
                    px2 = ps384.tile([128, D], F32, tag="ps384")
                    for m in range(12):
                        nc.tensor.matmul(px2, lhsT=ff1[:, m, ts(rt, 128)],
                                         rhs=w2_sb[:, m, :],
                                         start=(m == 0), stop=(m == 11))
                    nc.vector.tensor_add(xt[it], xt[it], px2)

            for l in range(nlayers):
                # ---- per-layer weights ----
                wq_sb = wqkv.tile([128, 3, D], F32R, tag="wq")
                nc.sync.dma_start(wq_sb, wq_d[l].rearrange("(kt p) n -> p kt n", p=128))
                wk_sb = wqkv.tile([128, 3, D], F32R, tag="wk")
                nc.sync.dma_start(wk_sb, wk_d[l].rearrange("(kt p) n -> p kt n", p=128))
                wv_sb = wqkv.tile([128, 3, D], F32R, tag="wv")
                nc.sync.dma_start(wv_sb, wv_d[l].rearrange("(kt p) n -> p kt n", p=128))
                wp_sb = wqkv.tile([128, 3, D], F32R, tag="wp")
                nc.sync.dma_start(wp_sb, wp_d[l].rearrange("(kt p) n -> p kt n", p=128))
                w1_sb = wff.tile([128, 3, FF], F32R, tag="w1")
                nc.sync.dma_start(w1_sb, w1_d[l].rearrange("(kt p) n -> p kt n", p=128))
                w2_sb = wff.tile([128, 12, D], F32R, tag="w2")
                nc.sync.dma_start(w2_sb, w2_d[l].rearrange("(kt p) n -> p kt n", p=128))
                b1_sb = wff.tile([128, 12], F32, tag="b1")
                nc.sync.dma_start(b1_sb, b1_d[l].rearrange("(m p) -> p m", p=128))

                for b in range(BL):
                    i0, i1 = 2 * b, 2 * b + 1
                    if do_attn:
                        attn_block(i0, i1, wq_sb, wk_sb, wv_sb, wp_sb)
                    if do_ff:
                        ff_block(i0, i1, w1_sb, w2_sb, b1_sb)

            # ---- final LN + lm head ----
            for b in range(BL):
                xfT = ln_transpose(2 * b, 2 * b + 1)
                pl = ps256.tile([V, 256], F32, tag="ps256")
                for kt in range(3):
                    nc.tensor.matmul(pl, lhsT=wlm_sb[:, kt, :], rhs=xfT[:, kt, :],
                                     start=(kt == 0), stop=(kt == 2))
                lg = work.tile([V, 256], F32, tag="lgs")
                nc.any.tensor_copy(lg, pl)
                nc.sync.dma_start(lg_d[:, ts(b, 256)], lg)

    nc.compile()
    return nc


def _prep_shared(tok_emb, pos_emb, Wq, Wk, Wv, Wproj, W1, b1, W2, Wlm):
    wq = (Wq.transpose(0, 2, 1, 3).reshape(L, D, D) * SCALE).astype(np.float32)
    wk = Wk.transpose(0, 2, 1, 3).reshape(L, D, D).astype(np.float32)
    wv = Wv.transpose(0, 2, 1, 3).reshape(L, D, D).astype(np.float32)
    emat = np.zeros((H, D), np.float32)
    for h in range(H):
        emat[h, HS * h:HS * (h + 1)] = 1.0
    masks = np.zeros((128, 2, 256), np.float32)
    for kt in range(2):
        u = kt * 128 + np.arange(128)[:, None]
        masks[:, kt, :] = np.where(np.arange(256)[None, :] >= u, 0.0, NEG)
    shared = {
        "wq": wq, "wk": wk, "wv": wv,
        "wp": np.ascontiguousarray(Wproj, dtype=np.float32),
        "w1": np.ascontiguousarray(W1, dtype=np.float32),
        "w2": np.ascontiguousarray(W2, dtype=np.float32),
        "b1": np.ascontiguousarray(b1, dtype=np.float32),
        "wlm": np.ascontiguousarray(Wlm, dtype=np.float32),
        "emat": emat,
        "ident": np.eye(128, dtype=np.float32),
        "masks": masks,
        "vones": np.ones((128, 2 * H), np.float32),
    }
    return shared


def _forward_numpy(idx, target, tok_emb, pos_emb, ln1_g, ln1_b, Wq, Wk, Wv,
                   Wproj, bproj, ln2_g, ln2_b, W1, b1, W2, b2, lnf_g, lnf_b,
                   Wlm, blm, nlayers=L, do_attn=True, do_ff=True):
    """Exact fp32/64 numpy forward (also partial, for debugging)."""
    def ln(x, g, b):
        m = x.mean(-1, keepdims=True)
        v = ((x - m) ** 2).mean(-1, keepdims=True)
        return (x - m) / np.sqrt(v + EPS) * g + b

    x = tok_emb[idx] + pos_emb[None, :idx.shape[1]]
    Tt = idx.shape[1]
    mask = np.tril(np.ones((Tt, Tt), bool))
    for l in range(nlayers):
        if do_attn:
            h = ln(x, ln1_g[l], ln1_b[l])
            q = np.einsum('btd,hde->bhte', h, Wq[l])
            k = np.einsum('btd,hde->bhte', h, Wk[l])
            v = np.einsum('btd,hde->bhte', h, Wv[l])
            s = np.einsum('bhte,bhse->bhts', q, k) * SCALE
            s = np.where(mask, s, -np.inf)
            s = s - s.max(-1, keepdims=True)
            e = np.exp(s)
            a = e / e.sum(-1, keepdims=True)
            o = np.einsum('bhts,bhse->bhte', a, v)
            o = o.transpose(0, 2, 1, 3).reshape(x.shape[0], Tt, D)
            x = x + o @ Wproj[l] + bproj[l]
        if do_ff:
            h2 = ln(x, ln2_g[l], ln2_b[l])
            x = x + np.maximum(h2 @ W1[l] + b1[l], 0.0) @ W2[l] + b2[l]
    x = ln(x, lnf_g, lnf_b)
    return (x @ Wlm + blm).reshape(-1, V)


def _loss_from_logits(logits, target):
    lg = logits.astype(np.float64)
    m = lg.max(axis=1, keepdims=True)
    lse = m[:, 0] + np.log(np.exp(lg - m).sum(axis=1))
    picked = lg[np.arange(lg.shape[0]), target.reshape(-1)]
    return np.float32((lse - picked).mean())


def _run_device(args, nc):
    from concourse.bass_utils import run_bass_kernel_spmd
    x0 = (args["tok_emb"][args["idx"]]
          + args["pos_emb"][None, :T]).astype(np.float32)      # [B, T, D]
    shared = _prep_shared(args["tok_emb"], args["pos_emb"], args["Wq"],
                          args["Wk"], args["Wv"], args["Wproj"], args["W1"],
                          args["b1"], args["W2"], args["Wlm"])
    in_maps = []
    for c in range(NCORES):
        m = dict(shared)
        m["x0"] = x0[c * BL:(c + 1) * BL].reshape(R, D)
        in_maps.append(m)
    res = run_bass_kernel_spmd(nc, in_maps, list(range(NCORES))).results
    logitsT = np.concatenate([res[c]["logitsT"] for c in range(NCORES)], axis=1)
    logits = np.ascontiguousarray(logitsT.T) + args["blm"][None, :]
    return logits.astype(np.float32)


def kernel(idx, target, tok_emb, pos_emb, ln1_g, ln1_b, Wq, Wk, Wv, Wproj,
           bproj, ln2_g, ln2_b, W1, b1, W2, b2, lnf_g, lnf_b, Wlm, blm):
    args = dict(idx=idx, target=target, tok_emb=tok_emb, pos_emb=pos_emb,
                ln1_g=ln1_g, ln1_b=ln1_b, Wq=Wq, Wk=Wk, Wv=Wv, Wproj=Wproj,
                bproj=bproj, ln2_g=ln2_g, ln2_b=ln2_b, W1=W1, b1=b1, W2=W2,
                b2=b2, lnf_g=lnf_g, lnf_b=lnf_b, Wlm=Wlm, blm=blm)
    args = {k: np.asarray(v) for k, v in args.items()}

    # The device kernel hardcodes gain=1/bias=0 layernorms and zero residual
    # biases (true for this model's setup).  Anything else -> exact fallback.
    trivial = (np.all(args["ln1_g"] == 1) and np.all(args["ln1_b"] == 0)
               and np.all(args["ln2_g"] == 1) and np.all(args["ln2_b"] == 0)
               and np.all(args["lnf_g"] == 1) and np.all(args["lnf_b"] == 0)
               and np.all(args["bproj"] == 0) and np.all(args["b2"] == 0))
    if not trivial:
        logits = _forward_numpy(**args).astype(np.float32)
        return logits, _loss_from_logits(logits, args["target"])

    if "nc" not in _CACHE:
        _CACHE["nc"] = _build_nc()
    logits = _run_device(args, _CACHE["nc"])
    loss = _loss_from_logits(logits, args["target"])
    return logits, loss


# revision 19
# speedup vs baseline: 1.0193x; 1.0193x over previous
"""Trainium2 Bass kernel for a 6-layer GPT forward pass (BigramLanguageModel).

Strategy: data-parallel over batch across 8 NeuronCores (16 samples/core),
no collectives.  Host does the embedding gather (pure input prep) and the
final loss reduction from the logits the device produces.

Device-side layout: residual stream x kept row-major [tokens, D] in SBUF
(exact fp32); LayerNorm outputs are PE-transposed to feature-major [D, tok]
chunks; q/k are produced directly transposed via matmul orientation;
attention scores are computed transposed [Tk, Tq] so softmax denominators
come for free from a ones-column appended to V; all matmuls in float32r
(1 cycle/row at N>=256).
"""
import sys
import numpy as np

sys.path.insert(0, '/opt/trn_rl_repo')

V, D, T, H, HS, L, B, FF = 65, 384, 256, 6, 64, 6, 128, 1536
EPS = 1e-5
SCALE = D ** -0.5
NCORES = 8
BL = B // NCORES            # local batch per core
R = BL * T                  # rows (tokens) per core
RTILES = R // 128           # 32 row tiles
NEG = -30.0                 # additive causal mask value (exp(-30) ~= 1e-13)

_CACHE = {}


def _build_nc(nlayers=L, do_attn=True, do_ff=True, repeat=1):
    import concourse.bacc as bacc
    import concourse.tile as tile
    from concourse import mybir
    from concourse.bass import ts

    F32 = mybir.dt.float32
    F32R = mybir.dt.float32r
    AF = mybir.ActivationFunctionType
    ALU = mybir.AluOpType

    nc = bacc.Bacc()

    x0_d = nc.declare_dram_parameter("x0", [R, D], F32, isOutput=False)
    wq_d = nc.declare_dram_parameter("wq", [L, D, D], F32R, isOutput=False)
    wk_d = nc.declare_dram_parameter("wk", [L, D, D], F32R, isOutput=False)
    wv_d = nc.declare_dram_parameter("wv", [L, D, D], F32R, isOutput=False)
    wp_d = nc.declare_dram_parameter("wp", [L, D, D], F32R, isOutput=False)
    w1_d = nc.declare_dram_parameter("w1", [L, D, FF], F32R, isOutput=False)
    w2_d = nc.declare_dram_parameter("w2", [L, FF, D], F32R, isOutput=False)
    b1_d = nc.declare_dram_parameter("b1", [L, FF], F32, isOutput=False)
    wlm_d = nc.declare_dram_parameter("wlm", [D, V], F32R, isOutput=False)
    em_d = nc.declare_dram_parameter("emat", [H, D], F32R, isOutput=False)
    id_d = nc.declare_dram_parameter("ident", [128, 128], F32R, isOutput=False)
    mk_d = nc.declare_dram_parameter("masks", [128, 2, 256], F32, isOutput=False)
    on_d = nc.declare_dram_parameter("vones", [128, 2 * H], F32R, isOutput=False)
    lg_d = nc.declare_dram_parameter("logitsT", [V, R], F32, isOutput=True)

    with tile.TileContext(nc) as tc:
        with (
            tc.tile_pool(name="xres", bufs=1) as xres,
            tc.tile_pool(name="const", bufs=1) as cpool,
            tc.tile_pool(name="wqkv", bufs=2) as wqkv,
            tc.tile_pool(name="wff", bufs=1) as wff,
            tc.tile_pool(name="work", bufs=2) as work,
            tc.tile_pool(name="small", bufs=6) as small,
            tc.tile_pool(name="htmp", bufs=4) as htmp_pool,
            tc.tile_pool(name="ff1p", bufs=1) as ff1p,
            tc.tile_pool(name="ps256", bufs=5, space="PSUM") as ps256,
            tc.tile_pool(name="psO", bufs=1, space="PSUM") as psOp,
        ):
            # ---- constants ----
            ident = cpool.tile([128, 128], F32R)
            nc.sync.dma_start(ident, id_d[:, :])
            masks = cpool.tile([128, 2, 256], F32)
            nc.sync.dma_start(masks, mk_d[:, :, :])
            emat = cpool.tile([H, D], F32R)
            nc.sync.dma_start(emat, em_d[:, :])
            vones = cpool.tile([128, 2 * H], F32R)
            nc.sync.dma_start(vones, on_d[:, :])
            wlm_sb = cpool.tile([128, 3, V], F32R)
            nc.sync.dma_start(wlm_sb, wlm_d.rearrange("(kt p) n -> p kt n", p=128))
            eps_t = cpool.tile([128, 1], F32)
            nc.vector.memset(eps_t, EPS)

            # ---- residual stream ----
            x0r = x0_d.rearrange("(n p) d -> n p d", p=128)
            xt = [None] * RTILES

            def load_x0():
                for i in range(RTILES):
                    xtile = xres.tile([128, D], F32, tag=f"x{i}")
                    nc.sync.dma_start(xtile, x0r[i])
                    xt[i] = xtile

            def pass_stats():
                """Batched LN stats for all row tiles: one Sqrt per pass so
                the ACT table stays on the exp set otherwise."""
                mvall = small.tile([128, RTILES, 2], F32, tag="mvall")
                for i in range(RTILES):
                    st = small.tile([128, 6], F32, tag="bnst")
                    nc.vector.bn_stats(out=st, in_=xt[i])
                    nc.vector.bn_aggr(out=mvall[:, i, :], in_=st)
                std = small.tile([128, RTILES], F32, tag="stdall")
                nc.scalar.activation(out=std, in_=mvall[:, :, 1], func=AF.Sqrt,
                                     bias=eps_t, scale=1.0)
                rstd = small.tile([128, RTILES], F32, tag="rstdall")
                nc.vector.reciprocal(out=rstd, in_=std)
                return mvall, rstd

            def layernorm(i, out_ap, stats):
                mvall, rstd = stats
                with nc.allow_low_precision(reason="f32r matmul inputs"):
                    nc.vector.tensor_scalar(
                        out=out_ap, in0=xt[i], scalar1=mvall[:, i, 0:1],
                        scalar2=rstd[:, i:i + 1],
                        op0=ALU.subtract, op1=ALU.mult)

            def ln_pair(i0, i1, stats):
                h0 = htmp_pool.tile([128, D], F32R, tag="htmp")
                layernorm(i0, h0, stats)
                h1 = htmp_pool.tile([128, D], F32R, tag="htmp")
                layernorm(i1, h1, stats)
                return h0, h1

            def transpose_pair(h01, tag="hT"):
                h0, h1 = h01
                hT = work.tile([128, 3, 256], F32R, tag=tag)
                for g in range(3):
                    pt = ps256.tile([128, 256], F32R, tag="ps256")
                    nc.tensor.transpose(pt[:, 0:128], h0[:, ts(g, 128)], ident)
                    nc.tensor.transpose(pt[:, 128:256], h1[:, ts(g, 128)], ident)
                    nc.scalar.activation(out=hT[:, g, :], in_=pt, func=AF.Copy)
                return hT

            def attn_block(i0, i1, hT, wq_sb, wk_sb, wv_sb, wp_sb):
                qT = work.tile([128, 3, 256], F32R, tag="qT")
                kT = work.tile([128, 3, 256], F32R, tag="kT")
                for dst, w_sb, cp in (
                        (qT, wq_sb, lambda o, i: nc.vector.tensor_copy(o, i)),
                        (kT, wk_sb,
                         lambda o, i: nc.scalar.activation(out=o, in_=i,
                                                           func=AF.Copy))):
                    for g in range(3):
                        pq = ps256.tile([128, 256], F32, tag="ps256")
                        for kt in range(3):
                            nc.tensor.matmul(pq, lhsT=w_sb[:, kt, ts(g, 128)],
                                             rhs=hT[:, kt, :],
                                             start=(kt == 0), stop=(kt == 2))
                        cp(dst[:, g, :], pq)
                vaug = work.tile([128, 2, H, 65], F32R, tag="vaug")
                nc.vector.tensor_copy(
                    vaug[:, :, :, 64:65].rearrange("p a h o -> p (a h o)"), vones)
                for tt in range(2):
                    pv = ps256.tile([128, D], F32, tag="ps256")
                    for kt in range(3):
                        nc.tensor.matmul(pv, lhsT=hT[:, kt, ts(tt, 128)],
                                         rhs=wv_sb[:, kt, :],
                                         start=(kt == 0), stop=(kt == 2))
                    nc.any.tensor_copy(
                        vaug[:, tt, :, 0:64],
                        pv.rearrange("p (h e) -> p h e", e=64))

                psO = psOp.tile([V, H, 256], F32, tag="psO")
                for h in range(H):
                    g, r0 = h // 2, 64 * (h % 2)
                    kT_h = kT[r0:r0 + 64, g, :]
                    qT_h = qT[r0:r0 + 64, g, :]
                    sT = work.tile([128, 2, 256], F32R, tag="sT")
                    pst = ps256.tile([128, 2, 256], F32, tag="ps256")
                    for kt in range(2):
                        nc.tensor.matmul(pst[:, kt, :], lhsT=kT_h[:, ts(kt, 128)],
                                         rhs=qT_h, start=True, stop=True)
                    with nc.allow_low_precision(reason="softmax f32r"):
                        nc.vector.tensor_add(
                            sT.rearrange("p a t -> p (a t)"),
                            pst.rearrange("p a t -> p (a t)"),
                            masks.rearrange("p a t -> p (a t)"))
                    nc.scalar.activation(
                        out=sT.rearrange("p a t -> p (a t)"),
                        in_=sT.rearrange("p a t -> p (a t)"), func=AF.Exp)
                    for kt in range(2):
                        nc.tensor.matmul(psO[:, h, :],
                                         lhsT=vaug[:, kt, h, :],
                                         rhs=sT[:, kt, :],
                                         start=(kt == 0), stop=(kt == 1))

                # denominators -> reciprocal -> broadcast via E-matmul
                rec1 = work.tile([1, H * 256], F32R, tag="rec1")
                with nc.allow_low_precision(reason="softmax denom"):
                    nc.vector.reciprocal(
                        out=rec1,
                        in_=psO[64:65, :, :].rearrange("o h t -> o (h t)"))
                rec = work.tile([H, 256], F32R, tag="rec")
                nc.sync.dma_start(rec, rec1[:, :])
                oT = work.tile([128, 3, 256], F32R, tag="oT")
                for g in range(3):
                    pbc = ps256.tile([128, 256], F32, tag="ps256")
                    nc.tensor.matmul(pbc, lhsT=emat[:, ts(g, 128)], rhs=rec,
                                     start=True, stop=True)
                    bc = work.tile([128, 256], F32, tag="bc")
                    nc.scalar.activation(out=bc, in_=pbc, func=AF.Copy)
                    with nc.allow_low_precision(reason="attn out f32r"):
                        nc.vector.tensor_mul(oT[0:64, g, :],
                                             psO[0:64, 2 * g, :], bc[0:64, :])
                        nc.vector.tensor_mul(oT[64:128, g, :],
                                             psO[0:64, 2 * g + 1, :],
                                             bc[64:128, :])

                # projection + residual
                for rt, it in ((0, i0), (1, i1)):
                    pxp = ps256.tile([128, D], F32, tag="ps256")
                    for g in range(3):
                        nc.tensor.matmul(pxp, lhsT=oT[:, g, ts(rt, 128)],
                                         rhs=wp_sb[:, g, :],
                                         start=(g == 0), stop=(g == 2))
                    nc.vector.tensor_add(xt[it], xt[it], pxp)

            def ff_block(h2T, i0, i1, w1_sb, w2_sb):
                ff1 = ff1p.tile([128, 12, 256], F32R, tag="ff1")
                for mp in range(6):
                    pf = ps256.tile([128, 2, 256], F32, tag="ps256")
                    for j in range(2):
                        m = 2 * mp + j
                        for kt in range(3):
                            nc.tensor.matmul(pf[:, j, :],
                                             lhsT=w1_sb[:, kt, ts(m, 128)],
                                             rhs=h2T[:, kt, :],
                                             start=(kt == 0), stop=(kt == 2))
                    nc.scalar.activation(
                        out=ff1[:, 2 * mp:2 * mp + 2, :].rearrange("p a t -> p (a t)"),
                        in_=pf.rearrange("p a t -> p (a t)"), func=AF.Relu)
                for rt, it in ((0, i0), (1, i1)):
                    px2 = ps256.tile([128, D], F32, tag="ps256")
                    for m in range(12):
                        nc.tensor.matmul(px2, lhsT=ff1[:, m, ts(rt, 128)],
                                         rhs=w2_sb[:, m, :],
                                         start=(m == 0), stop=(m == 11))
                    nc.vector.tensor_add(xt[it], xt[it], px2)

            for rep in range(repeat):
              load_x0()
              for l in range(nlayers):
                # ---- per-layer weights ----
                wq_sb = wqkv.tile([128, 3, D], F32R, tag="wq")
                nc.sync.dma_start(wq_sb, wq_d[l].rearrange("(kt p) n -> p kt n", p=128))
                wk_sb = wqkv.tile([128, 3, D], F32R, tag="wk")
                nc.sync.dma_start(wk_sb, wk_d[l].rearrange("(kt p) n -> p kt n", p=128))
                wv_sb = wqkv.tile([128, 3, D], F32R, tag="wv")
                nc.sync.dma_start(wv_sb, wv_d[l].rearrange("(kt p) n -> p kt n", p=128))
                wp_sb = wqkv.tile([128, 3, D], F32R, tag="wp")
                nc.sync.dma_start(wp_sb, wp_d[l].rearrange("(kt p) n -> p kt n", p=128))
                w1_sb = wff.tile([128, 3, FF], F32R, tag="w1")
                nc.sync.dma_start(w1_sb, w1_d[l].rearrange("(kt p) n -> p kt n", p=128))
                w2_sb = wff.tile([128, 12, D], F32R, tag="w2")
                nc.sync.dma_start(w2_sb, w2_d[l].rearrange("(kt p) n -> p kt n", p=128))

                if do_attn:
                    stats = pass_stats()
                    lns = {0: ln_pair(0, 1, stats)}
                    if BL > 1:
                        lns[1] = ln_pair(2, 3, stats)
                    hTs = {0: transpose_pair(lns.pop(0))}
                    for b in range(BL):
                        if b + 2 < BL:
                            lns[b + 2] = ln_pair(2 * b + 4, 2 * b + 5, stats)
                        if b + 1 < BL:
                            hTs[b + 1] = transpose_pair(lns.pop(b + 1))
                        attn_block(2 * b, 2 * b + 1, hTs.pop(b),
                                   wq_sb, wk_sb, wv_sb, wp_sb)
                if do_ff:
                    stats = pass_stats()
                    lns = {0: ln_pair(0, 1, stats)}
                    if BL > 1:
                        lns[1] = ln_pair(2, 3, stats)
                    hTs = {0: transpose_pair(lns.pop(0))}
                    for b in range(BL):
                        if b + 2 < BL:
                            lns[b + 2] = ln_pair(2 * b + 4, 2 * b + 5, stats)
                        if b + 1 < BL:
                            hTs[b + 1] = transpose_pair(lns.pop(b + 1))
                        ff_block(hTs.pop(b), 2 * b, 2 * b + 1, w1_sb, w2_sb)

              # ---- final LN + lm head ----
              stats = pass_stats()
              lns = {0: ln_pair(0, 1, stats)}
              if BL > 1:
                  lns[1] = ln_pair(2, 3, stats)
              hTs = {0: transpose_pair(lns.pop(0))}
              for b in range(BL):
                  if b + 2 < BL:
                      lns[b + 2] = ln_pair(2 * b + 4, 2 * b + 5, stats)
                  if b + 1 < BL:
                      hTs[b + 1] = transpose_pair(lns.pop(b + 1))
                  xfT = hTs.pop(b)
                  pl = ps256.tile([V, 256], F32, tag="ps256")
                  for kt in range(3):
                      nc.tensor.matmul(pl, lhsT=wlm_sb[:, kt, :], rhs=xfT[:, kt, :],
                                       start=(kt == 0), stop=(kt == 2))
                  lg = work.tile([V, 256], F32, tag="lgs")
                  nc.any.tensor_copy(lg, pl)
                  nc.sync.dma_start(lg_d[:, ts(b, 256)], lg)

    nc.compile()
    return nc


def _prep_shared(tok_emb, pos_emb, Wq, Wk, Wv, Wproj, W1, b1, W2, Wlm):
    wq = (Wq.transpose(0, 2, 1, 3).reshape(L, D, D) * SCALE).astype(np.float32)
    wk = Wk.transpose(0, 2, 1, 3).reshape(L, D, D).astype(np.float32)
    wv = Wv.transpose(0, 2, 1, 3).reshape(L, D, D).astype(np.float32)
    emat = np.zeros((H, D), np.float32)
    for h in range(H):
        emat[h, HS * h:HS * (h + 1)] = 1.0
    masks = np.zeros((128, 2, 256), np.float32)
    for kt in range(2):
        u = kt * 128 + np.arange(128)[:, None]
        masks[:, kt, :] = np.where(np.arange(256)[None, :] >= u, 0.0, NEG)
    shared = {
        "wq": wq, "wk": wk, "wv": wv,
        "wp": np.ascontiguousarray(Wproj, dtype=np.float32),
        "w1": np.ascontiguousarray(W1, dtype=np.float32),
        "w2": np.ascontiguousarray(W2, dtype=np.float32),
        "b1": np.ascontiguousarray(b1, dtype=np.float32),
        "wlm": np.ascontiguousarray(Wlm, dtype=np.float32),
        "emat": emat,
        "ident": np.eye(128, dtype=np.float32),
        "masks": masks,
        "vones": np.ones((128, 2 * H), np.float32),
    }
    return shared


def _forward_numpy(idx, target, tok_emb, pos_emb, ln1_g, ln1_b, Wq, Wk, Wv,
                   Wproj, bproj, ln2_g, ln2_b, W1, b1, W2, b2, lnf_g, lnf_b,
                   Wlm, blm, nlayers=L, do_attn=True, do_ff=True):
    """Exact fp32/64 numpy forward (also partial, for debugging)."""
    def ln(x, g, b):
        m = x.mean(-1, keepdims=True)
        v = ((x - m) ** 2).mean(-1, keepdims=True)
        return (x - m) / np.sqrt(v + EPS) * g + b

    x = tok_emb[idx] + pos_emb[None, :idx.shape[1]]
    Tt = idx.shape[1]
    mask = np.tril(np.ones((Tt, Tt), bool))
    for l in range(nlayers):
        if do_attn:
            h = ln(x, ln1_g[l], ln1_b[l])
            q = np.einsum('btd,hde->bhte', h, Wq[l])
            k = np.einsum('btd,hde->bhte', h, Wk[l])
            v = np.einsum('btd,hde->bhte', h, Wv[l])
            s = np.einsum('bhte,bhse->bhts', q, k) * SCALE
            s = np.where(mask, s, -np.inf)
            s = s - s.max(-1, keepdims=True)
            e = np.exp(s)
            a = e / e.sum(-1, keepdims=True)
            o = np.einsum('bhts,bhse->bhte', a, v)
            o = o.transpose(0, 2, 1, 3).reshape(x.shape[0], Tt, D)
            x = x + o @ Wproj[l] + bproj[l]
        if do_ff:
            h2 = ln(x, ln2_g[l], ln2_b[l])
            x = x + np.maximum(h2 @ W1[l] + b1[l], 0.0) @ W2[l] + b2[l]
    x = ln(x, lnf_g, lnf_b)
    return (x @ Wlm + blm).reshape(-1, V)


def _loss_from_logits(logits, target):
    lg = logits.astype(np.float64)
    m = lg.max(axis=1, keepdims=True)
    lse = m[:, 0] + np.log(np.exp(lg - m).sum(axis=1))
    picked = lg[np.arange(lg.shape[0]), target.reshape(-1)]
    return np.float32((lse - picked).mean())


def _run_device(args, nc):
    from concourse.bass_utils import run_bass_kernel_spmd
    x0 = (args["tok_emb"][args["idx"]]
          + args["pos_emb"][None, :T]).astype(np.float32)      # [B, T, D]
    shared = _prep_shared(args["tok_emb"], args["pos_emb"], args["Wq"],
                          args["Wk"], args["Wv"], args["Wproj"], args["W1"],
                          args["b1"], args["W2"], args["Wlm"])
    in_maps = []
    for c in range(NCORES):
        m = dict(shared)
        m["x0"] = x0[c * BL:(c + 1) * BL].reshape(R, D)
        in_maps.append(m)
    res = run_bass_kernel_spmd(nc, in_maps, list(range(NCORES))).results
    logitsT = np.concatenate([res[c]["logitsT"] for c in range(NCORES)], axis=1)
    logits = np.ascontiguousarray(logitsT.T) + args["blm"][None, :]
    return logits.astype(np.float32)


def kernel(idx, target, tok_emb, pos_emb, ln1_g, ln1_b, Wq, Wk, Wv, Wproj,
           bproj, ln2_g, ln2_b, W1, b1, W2, b2, lnf_g, lnf_b, Wlm, blm):
    args = dict(idx=idx, target=target, tok_emb=tok_emb, pos_emb=pos_emb,
                ln1_g=ln1_g, ln1_b=ln1_b, Wq=Wq, Wk=Wk, Wv=Wv, Wproj=Wproj,
                bproj=bproj, ln2_g=ln2_g, ln2_b=ln2_b, W1=W1, b1=b1, W2=W2,
                b2=b2, lnf_g=lnf_g, lnf_b=lnf_b, Wlm=Wlm, blm=blm)
    args = {k: np.asarray(v) for k, v in args.items()}

    # The device kernel hardcodes gain=1/bias=0 layernorms and zero residual
    # biases (true for this model's setup).  Anything else -> exact fallback.
    trivial = (np.all(args["ln1_g"] == 1) and np.all(args["ln1_b"] == 0)
               and np.all(args["ln2_g"] == 1) and np.all(args["ln2_b"] == 0)
               and np.all(args["lnf_g"] == 1) and np.all(args["lnf_b"] == 0)
               and np.all(args["bproj"] == 0) and np.all(args["b2"] == 0)
               and np.all(args["b1"] == 0))
    if not trivial:
        logits = _forward_numpy(**args).astype(np.float32)
        return logits, _loss_from_logits(logits, args["target"])

    if "nc" not in _CACHE:
        _CACHE["nc"] = _build_nc()
    logits = _run_device(args, _CACHE["nc"])
    loss = _loss_from_logits(logits, args["target"])
    return logits, loss
